# revision 17
# baseline (speedup 1.0000x reference)
"""Distributed multi-head attention kernel for 8 TRN2 NeuronCores.

Sharding: 8-way head parallel (2 heads per core), batches looped on-core.
Each core: QKV projection for its 2 heads over both batches, per-head
attention (softmax without max-subtraction — logits are small; denominators
come from a ones-column appended to V so they fall out of the attn@V
matmul), then per-head AllToAlls across all 8 cores exchange row-blocks
for head-blocks: block s = (batch s//4, rows-block s%4).  Core c ends up
with all 1024 inner dims for (batch c//4, rows [(c%4)*512, ...)) and runs
the full output projection + bias on that slice.  The head-0 A2A fires at
attention half-time and overlaps head-1 compute; the exchange moves 1MB
of bf16 per core instead of all-reducing 8.4MB of fp32.

x and the weights are cast to bf16 on the host (bf16 is the compute
precision anyway), so there are no on-chip staging casts.  All transposes
run on the PE array (XBAR transposes proved slow for narrow blocks and
corrupt data when issued on two HWDGE queues concurrently).

Program order interleaves qkv of batch 1 with attention of (head 0,
batch 0) so the Scalar engine (exp — the second-busiest engine) starts
~45us in instead of after all projections.

The per-core output is the TRANSPOSED final slice [1024, 512] (PSUM-major
writes stay contiguous); the host transposes during assembly.
"""
import numpy as np

import concourse.bass as bass
import concourse.mybir as mybir
from concourse import bacc
import concourse.tile as tile
from concourse.bass_utils import run_bass_kernel_spmd
from concourse.masks import make_identity

# problem constants (hardcoded; kernel.py must be self-contained)
B, N, DIM = 2, 2048, 1024
H, DH = 16, 64
INNER = H * DH            # 1024
SCALE = DIM ** -0.5       # 1/32  (module scales by dim**-0.5, not dim_head)
NCORES = 8
HPC = H // NCORES         # 2 heads per core
SH = HPC * DH             # 128 inner cols per core
ROWS = N // 4             # 512 output rows per core
P = 128
KO = DIM // P             # 8 contraction chunks
JC = N // P               # 16 row chunks
IB = 512                  # query block size
NIB = N // IB             # 4 query blocks
ISUB = IB // P            # 4
FP32 = mybir.dt.float32
BF16 = mybir.dt.bfloat16

REPLICA_GROUPS = [[0, 1, 2, 3, 4, 5, 6, 7]]

_NC_CACHE = {}

# set by the last kernel() call when BASS_KERNEL_TRACE=1 (for test.py)
LAST_RESULTS = None


def _build():
    nc = bacc.Bacc(num_devices=NCORES)

    x_ext = nc.declare_dram_parameter("x", [B * N, DIM], BF16, isOutput=False)
    wq_ext = nc.declare_dram_parameter("wq", [DIM, SH], BF16, isOutput=False)
    wk_ext = nc.declare_dram_parameter("wk", [DIM, SH], BF16, isOutput=False)
    wv_ext = nc.declare_dram_parameter("wv", [DIM, SH], BF16, isOutput=False)
    wo_ext = nc.declare_dram_parameter("wo", [DIM, DIM], BF16, isOutput=False)
    bo_ext = nc.declare_dram_parameter("bo", [DIM], FP32, isOutput=False)
    out_ext = nc.declare_dram_parameter("out", [DIM, ROWS], FP32, isOutput=True)

    with tile.TileContext(nc) as tc:
        with (
            tc.tile_pool(name="consts", bufs=1) as consts,
            tc.tile_pool(name="stage", bufs=3) as stage,
            tc.tile_pool(name="xt_pool", bufs=2) as xt_pool,
            tc.tile_pool(name="pt_pool", bufs=2) as pt_pool,
            tc.tile_pool(name="nrm", bufs=4) as nrm,
            tc.tile_pool(name="dram", bufs=1, space="DRAM") as dram,
            tc.tile_pool(name="st_psum", bufs=3, space="PSUM") as st_psum,
            tc.tile_pool(name="o_psum", bufs=2, space="PSUM") as o_psum,
        ):
            ident_bf = consts.tile([P, P], BF16)
            make_identity(nc, ident_bf)

            # persistent tensors
            wq_sb = consts.tile([P, KO, SH], BF16)
            wk_sb = consts.tile([P, KO, SH], BF16)
            wv_sb = consts.tile([P, KO, SH], BF16)
            wo_sb = consts.tile([P, KO, DIM], BF16)
            bias_sb = consts.tile([P, KO], FP32)
            qT = consts.tile([P, B, N], BF16)    # [h*64+d, b, i]
            kT = consts.tile([P, B, N], BF16)
            v_aug = consts.tile([P, B, JC, HPC, DH + 1], BF16)
            out_rows = consts.tile([P, B, JC, SH], BF16)
            attnT = xt_pool.tile([P, KO, N], BF16, tag="xT", name="attnT")[
                :, :, :ROWS
            ]

            a2a_in0 = dram.tile([NCORES, NIB, P, DH], BF16, name="a2a_in0")
            a2a_in1 = dram.tile([NCORES, NIB, P, DH], BF16, name="a2a_in1")
            a2a_out0 = dram.tile([NCORES, ROWS, DH], BF16, name="a2a_out0")
            a2a_out1 = dram.tile([NCORES, ROWS, DH], BF16, name="a2a_out1")
            a2a_ins = [a2a_in0, a2a_in1]
            a2a_outs = [a2a_out0, a2a_out1]

            nc.vector.memset(v_aug[:, :, :, :, DH : DH + 1], 1.0)

            def load_x_and_transpose(b):
                """x[b] -> xT (PE transposes; straight bf16 loads)."""
                xT = xt_pool.tile([P, KO, N], BF16, tag="xT", name="xT")
                for mt in range(JC):
                    xst = stage.tile([P, DIM], BF16, tag="xst", name="xst")
                    nc.sync.dma_start(
                        xst, x_ext[b * N + mt * P : b * N + (mt + 1) * P, :]
                    )
                    for kg in range(2):
                        tp_ps = o_psum.tile(
                            [P, 4, P], BF16, tag="po", name="tp_ps"
                        )
                        for q in range(4):
                            ko = kg * 4 + q
                            nc.tensor.transpose(
                                tp_ps[:, q, :],
                                xst[:, ko * P : (ko + 1) * P],
                                ident_bf,
                            )
                        nc.vector.tensor_copy(
                            xT[:, kg * 4 : (kg + 1) * 4, mt * P : (mt + 1) * P],
                            tp_ps,
                        )
                return xT

            def load_qkv_weights():
                for w_ext, w_sb in (
                    (wk_ext, wk_sb),
                    (wq_ext, wq_sb),
                    (wv_ext, wv_sb),
                ):
                    nc.sync.dma_start(
                        w_sb, w_ext.rearrange("(ko kp) c -> kp ko c", kp=P)
                    )

            def load_out_weights():
                nc.sync.dma_start(
                    wo_sb, wo_ext.rearrange("(ko kp) c -> kp ko c", kp=P)
                )
                nc.sync.dma_start(
                    bias_sb, bo_ext.rearrange("(co cp) -> cp co", cp=P)
                )

            def qkv_proj(b, xT):
                # k first (attention's dots consume kT earliest), then q, v
                for w_sb, dstT in ((wk_sb, kT), (wq_sb, qT)):
                    for nb in range(NIB):
                        ps2 = st_psum.tile(
                            [P, 2, IB], FP32, tag="st", name="qk_ps"
                        )
                        ps = ps2[:, 0, :]
                        for ko in range(KO):
                            nc.tensor.matmul(
                                ps,
                                w_sb[:, ko, :],
                                xT[:, ko, nb * IB : (nb + 1) * IB],
                                start=(ko == 0),
                                stop=(ko == KO - 1),
                            )
                        nc.vector.tensor_copy(
                            dstT[:, b, nb * IB : (nb + 1) * IB], ps
                        )
                for mt in range(JC):
                    psv2 = st_psum.tile([P, 2, IB], FP32, tag="st", name="v_ps")
                    ps_v = psv2[:, 0, :SH]
                    for ko in range(KO):
                        nc.tensor.matmul(
                            ps_v,
                            xT[:, ko, mt * P : (mt + 1) * P],
                            wv_sb[:, ko, :],
                            start=(ko == 0),
                            stop=(ko == KO - 1),
                        )
                    nc.vector.tensor_copy(
                        v_aug[:, b, mt, :, 0:DH],
                        ps_v.rearrange("p (h d) -> p h d", d=DH),
                    )

            def attention(h, b):
                """ST = k@q.T per j-chunk, exp on ACT, attn@V with the
                denominator in column DH.  All 4 i-sub accumulations of a
                block share one PSUM bank so the 'po' slots rotate once
                per block, not once per i-sub (normalizes are emitted
                after all 64 matmuls so the bank tracker doesn't
                interleave PE writes with DVE reads)."""
                po = h * DH
                for ib in range(NIB):
                    ptile = pt_pool.tile([P, JC, IB], BF16, tag="pt", name="ptile")
                    for jg in range(JC // 2):
                        ps_st = st_psum.tile(
                            [P, 2, IB], FP32, tag="st", name="st_ps"
                        )
                        for u in range(2):
                            jc = jg * 2 + u
                            nc.tensor.matmul(
                                ps_st[:, u, :],
                                kT[po : po + DH, b, jc * P : (jc + 1) * P],
                                qT[po : po + DH, b, ib * IB : (ib + 1) * IB],
                                start=True,
                                stop=True,
                            )
                        nc.scalar.activation(
                            ptile[:, jg * 2 : (jg + 1) * 2, :],
                            ps_st,
                            mybir.ActivationFunctionType.Exp,
                            scale=SCALE,
                        )
                    ps_o4 = o_psum.tile(
                        [P, ISUB, DH + 1], FP32, tag="po", name="o_ps"
                    )
                    for isub in range(ISUB):
                        for jc in range(JC):
                            nc.tensor.matmul(
                                ps_o4[:, isub, :],
                                ptile[:, jc, isub * P : (isub + 1) * P],
                                v_aug[:, b, jc, h, :],
                                start=(jc == 0),
                                stop=(jc == JC - 1),
                            )
                    for isub in range(ISUB):
                        ic = ib * ISUB + isub
                        recip = nrm.tile([P, 1], FP32, tag="recip", name="recip")
                        nc.vector.reciprocal(recip, ps_o4[:, isub, DH : DH + 1])
                        nc.vector.tensor_scalar_mul(
                            out_rows[:, b, ic, po : po + DH],
                            ps_o4[:, isub, 0:DH],
                            recip,
                        )
                    # block (h, b, ib) complete -> stage its A2A input
                    s = b * NIB + ib
                    nc.sync.dma_start(
                        a2a_ins[h][s].rearrange("ic p c -> p ic c"),
                        out_rows[
                            :, b, ib * ISUB : (ib + 1) * ISUB, po : po + DH
                        ],
                    )

            def a2a_exchange(h):
                nc.gpsimd.collective_compute(
                    "AllToAll",
                    mybir.AluOpType.bypass,
                    replica_groups=REPLICA_GROUPS,
                    ins=[a2a_ins[h].opt()],
                    outs=[a2a_outs[h].opt()],
                )

            def receive(h):
                po = h * DH
                for i in range(NCORES):
                    rstage = pt_pool.tile(
                        [P, NIB, DH], BF16, tag="rstage", name="rstage"
                    )
                    nc.sync.dma_start(
                        rstage,
                        a2a_outs[h][i].rearrange("(ic p) c -> p ic c", p=P),
                    )
                    rps = st_psum.tile([DH, NIB, P], BF16, tag="st", name="r_ps")
                    for q in range(NIB):
                        nc.tensor.transpose(rps[:, q, :], rstage[:, q, :], ident_bf)
                    nc.vector.tensor_copy(attnT[po : po + DH, i, :], rps)

            def final_projection():
                for cc in range(KO):
                    psf2 = st_psum.tile([P, 2, IB], FP32, tag="st", name="f_ps")
                    ps_f = psf2[:, 0, :ROWS]
                    for ko in range(KO):
                        nc.tensor.matmul(
                            ps_f,
                            wo_sb[:, ko, cc * P : (cc + 1) * P],
                            attnT[:, ko, :],
                            start=(ko == 0),
                            stop=(ko == KO - 1),
                        )
                    of = stage.tile([P, ROWS], FP32, tag="of", name="of")
                    nc.vector.tensor_scalar_add(of, ps_f, bias_sb[:, cc : cc + 1])
                    nc.sync.dma_start(out_ext[cc * P : (cc + 1) * P, :], of)

            # ---- program order chosen so exp starts early and the PE
            # always has lower-priority projection work to fill gaps ----
            load_qkv_weights()
            xT0 = load_x_and_transpose(0)
            qkv_proj(0, xT0)
            attention(0, 0)
            # batch-1 x^T via XBAR DMA transpose (single queue — safe); the
            # PE is busy with attention(0,0) so spending DMA instead of PE
            # cycles here shortens the critical path
            xT1 = xt_pool.tile([P, KO, N], BF16, tag="xT", name="xT1")
            for ko in range(KO):
                nc.sync.dma_start_transpose(
                    xT1[:, ko, :],
                    x_ext[1 * N : 2 * N, ko * P : (ko + 1) * P],
                )
            load_out_weights()
            qkv_proj(1, xT1)
            attention(0, 1)
            a2a_exchange(0)        # flies under head-1 attention
            attention(1, 0)
            attention(1, 1)
            receive(0)             # PE work during the head-1 A2A flight
            a2a_exchange(1)
            receive(1)
            final_projection()

    nc.finalize()
    return nc


def _get_nc():
    if "nc" not in _NC_CACHE:
        _NC_CACHE["nc"] = _build()
    return _NC_CACHE["nc"]


def kernel(**inputs) -> np.ndarray:
    import os

    import ml_dtypes

    global LAST_RESULTS

    bf16 = ml_dtypes.bfloat16
    x = np.asarray(inputs["x"], dtype=np.float32)
    W_qkv = np.asarray(inputs["W_qkv"], dtype=np.float32)
    W_out = np.asarray(inputs["W_out"], dtype=np.float32)
    b_out = np.ascontiguousarray(np.asarray(inputs["b_out"], dtype=np.float32))

    x_bf = np.ascontiguousarray(x.reshape(B * N, DIM).astype(bf16))
    wo_bf = np.ascontiguousarray(W_out.astype(bf16))
    wqkv_bf = W_qkv.astype(bf16)

    nc = _get_nc()

    in_maps = []
    for c in range(NCORES):
        in_maps.append(
            {
                "x": x_bf,
                "wq": np.ascontiguousarray(
                    wqkv_bf[:, 0 * INNER + c * SH : 0 * INNER + (c + 1) * SH]
                ),
                "wk": np.ascontiguousarray(
                    wqkv_bf[:, 1 * INNER + c * SH : 1 * INNER + (c + 1) * SH]
                ),
                "wv": np.ascontiguousarray(
                    wqkv_bf[:, 2 * INNER + c * SH : 2 * INNER + (c + 1) * SH]
                ),
                "wo": wo_bf,
                "bo": b_out,
            }
        )

    trace = os.environ.get("BASS_KERNEL_TRACE", "0") == "1"
    res = run_bass_kernel_spmd(
        nc, in_maps, core_ids=list(range(NCORES)), trace=trace
    )
    LAST_RESULTS = res

    y = np.empty((B, N, DIM), dtype=np.float32)
    for c in range(NCORES):
        b, r = c // 4, c % 4
        y[b, r * ROWS : (r + 1) * ROWS, :] = res.results[c]["out"].T
    return y


# revision 18
# speedup vs baseline: 1.0594x; 1.0594x over previous
"""Distributed multi-head attention kernel for 8 TRN2 NeuronCores.

Sharding: 8-way head parallel (2 heads per core), batches looped on-core.
Each core: QKV projection for its 2 heads over both batches, per-head
attention (softmax without max-subtraction — logits are small; denominators
come from a ones-column appended to V so they fall out of the attn@V
matmul), then per-head AllToAlls across all 8 cores exchange row-blocks
for head-blocks: block s = (batch s//4, rows-block s%4).  Core c ends up
with all 1024 inner dims for (batch c//4, rows [(c%4)*512, ...)) and runs
the full output projection + bias on that slice.  The head-0 A2A fires at
attention half-time and overlaps head-1 compute; the exchange moves 1MB
of bf16 per core instead of all-reducing 8.4MB of fp32.

x and the weights are cast to bf16 on the host (bf16 is the compute
precision anyway), so there are no on-chip staging casts.  All transposes
run on the PE array (XBAR transposes proved slow for narrow blocks and
corrupt data when issued on two HWDGE queues concurrently).

Program order interleaves qkv of batch 1 with attention of (head 0,
batch 0) so the Scalar engine (exp — the second-busiest engine) starts
~45us in instead of after all projections.

The per-core output is the TRANSPOSED final slice [1024, 512] (PSUM-major
writes stay contiguous); the host transposes during assembly.
"""
import numpy as np

import concourse.bass as bass
import concourse.mybir as mybir
from concourse import bacc
import concourse.tile as tile
from concourse.bass_utils import run_bass_kernel_spmd
from concourse.masks import make_identity

# problem constants (hardcoded; kernel.py must be self-contained)
B, N, DIM = 2, 2048, 1024
H, DH = 16, 64
INNER = H * DH            # 1024
SCALE = DIM ** -0.5       # 1/32  (module scales by dim**-0.5, not dim_head)
NCORES = 8
HPC = H // NCORES         # 2 heads per core
SH = HPC * DH             # 128 inner cols per core
ROWS = N // 4             # 512 output rows per core
P = 128
KO = DIM // P             # 8 contraction chunks
JC = N // P               # 16 row chunks
IB = 512                  # query block size
NIB = N // IB             # 4 query blocks
ISUB = IB // P            # 4
FP32 = mybir.dt.float32
BF16 = mybir.dt.bfloat16

REPLICA_GROUPS = [[0, 1, 2, 3, 4, 5, 6, 7]]

_NC_CACHE = {}

# set by the last kernel() call when BASS_KERNEL_TRACE=1 (for test.py)
LAST_RESULTS = None


def _build():
    nc = bacc.Bacc(num_devices=NCORES)

    x_ext = nc.declare_dram_parameter("x", [B * N, DIM], BF16, isOutput=False)
    wq_ext = nc.declare_dram_parameter("wq", [DIM, SH], BF16, isOutput=False)
    wk_ext = nc.declare_dram_parameter("wk", [DIM, SH], BF16, isOutput=False)
    wv_ext = nc.declare_dram_parameter("wv", [DIM, SH], BF16, isOutput=False)
    wo_ext = nc.declare_dram_parameter("wo", [DIM, DIM], BF16, isOutput=False)
    bo_ext = nc.declare_dram_parameter("bo", [DIM], FP32, isOutput=False)
    out_ext = nc.declare_dram_parameter("out", [DIM, ROWS], FP32, isOutput=True)

    with tile.TileContext(nc) as tc:
        with (
            tc.tile_pool(name="consts", bufs=1) as consts,
            tc.tile_pool(name="stage", bufs=3) as stage,
            tc.tile_pool(name="xt_pool", bufs=2) as xt_pool,
            tc.tile_pool(name="pt_pool", bufs=2) as pt_pool,
            tc.tile_pool(name="nrm", bufs=4) as nrm,
            tc.tile_pool(name="dram", bufs=1, space="DRAM") as dram,
            tc.tile_pool(name="st_psum", bufs=3, space="PSUM") as st_psum,
            tc.tile_pool(name="o_psum", bufs=2, space="PSUM") as o_psum,
        ):
            ident_bf = consts.tile([P, P], BF16)
            make_identity(nc, ident_bf)

            # persistent tensors
            wq_sb = consts.tile([P, KO, SH], BF16)
            wk_sb = consts.tile([P, KO, SH], BF16)
            wv_sb = consts.tile([P, KO, SH], BF16)
            wo_sb = consts.tile([P, KO, DIM], BF16)
            bias_sb = consts.tile([P, KO], FP32)
            qT = consts.tile([P, B, N], BF16)    # [h*64+d, b, i]
            kT = consts.tile([P, B, N], BF16)
            v_aug = consts.tile([P, B, JC, HPC, DH + 1], BF16)
            out_rows = consts.tile([P, B, JC, SH], BF16)
            attnT = xt_pool.tile([P, KO, N], BF16, tag="xT", name="attnT")[
                :, :, :ROWS
            ]

            a2a_in0 = dram.tile([NCORES, NIB, P, DH], BF16, name="a2a_in0")
            a2a_in1 = dram.tile([NCORES, NIB, P, DH], BF16, name="a2a_in1")
            a2a_out0 = dram.tile([NCORES, ROWS, DH], BF16, name="a2a_out0")
            a2a_out1 = dram.tile([NCORES, ROWS, DH], BF16, name="a2a_out1")
            a2a_ins = [a2a_in0, a2a_in1]
            a2a_outs = [a2a_out0, a2a_out1]

            nc.vector.memset(v_aug[:, :, :, :, DH : DH + 1], 1.0)

            def load_x_and_transpose(b):
                """x[b] -> xT (PE transposes; straight bf16 loads)."""
                xT = xt_pool.tile([P, KO, N], BF16, tag="xT", name="xT")
                for mt in range(JC):
                    xst = stage.tile([P, DIM], BF16, tag="xst", name="xst")
                    nc.sync.dma_start(
                        xst, x_ext[b * N + mt * P : b * N + (mt + 1) * P, :]
                    )
                    for kg in range(2):
                        tp_ps = o_psum.tile(
                            [P, 4, P], BF16, tag="po", name="tp_ps"
                        )
                        for q in range(4):
                            ko = kg * 4 + q
                            nc.tensor.transpose(
                                tp_ps[:, q, :],
                                xst[:, ko * P : (ko + 1) * P],
                                ident_bf,
                            )
                        nc.vector.tensor_copy(
                            xT[:, kg * 4 : (kg + 1) * 4, mt * P : (mt + 1) * P],
                            tp_ps,
                        )
                return xT

            def load_qkv_weights():
                for w_ext, w_sb in (
                    (wk_ext, wk_sb),
                    (wq_ext, wq_sb),
                    (wv_ext, wv_sb),
                ):
                    nc.sync.dma_start(
                        w_sb, w_ext.rearrange("(ko kp) c -> kp ko c", kp=P)
                    )

            def load_out_weights():
                nc.sync.dma_start(
                    wo_sb, wo_ext.rearrange("(ko kp) c -> kp ko c", kp=P)
                )
                nc.sync.dma_start(
                    bias_sb, bo_ext.rearrange("(co cp) -> cp co", cp=P)
                )

            def qkv_proj(b, xT):
                # k first (attention's dots consume kT earliest), then q, v
                for w_sb, dstT in ((wk_sb, kT), (wq_sb, qT)):
                    for nb in range(NIB):
                        ps2 = st_psum.tile(
                            [P, 2, IB], FP32, tag="st", name="qk_ps"
                        )
                        ps = ps2[:, 0, :]
                        for ko in range(KO):
                            nc.tensor.matmul(
                                ps,
                                w_sb[:, ko, :],
                                xT[:, ko, nb * IB : (nb + 1) * IB],
                                start=(ko == 0),
                                stop=(ko == KO - 1),
                            )
                        nc.vector.tensor_copy(
                            dstT[:, b, nb * IB : (nb + 1) * IB], ps
                        )
                for mt in range(JC):
                    psv2 = st_psum.tile([P, 2, IB], FP32, tag="st", name="v_ps")
                    ps_v = psv2[:, 0, :SH]
                    for ko in range(KO):
                        nc.tensor.matmul(
                            ps_v,
                            xT[:, ko, mt * P : (mt + 1) * P],
                            wv_sb[:, ko, :],
                            start=(ko == 0),
                            stop=(ko == KO - 1),
                        )
                    nc.vector.tensor_copy(
                        v_aug[:, b, mt, :, 0:DH],
                        ps_v.rearrange("p (h d) -> p h d", d=DH),
                    )

            def attention(h, b):
                """ST = k@q.T per j-chunk, exp on ACT, attn@V with the
                denominator in column DH.  All 4 i-sub accumulations of a
                block share one PSUM bank so the 'po' slots rotate once
                per block, not once per i-sub (normalizes are emitted
                after all 64 matmuls so the bank tracker doesn't
                interleave PE writes with DVE reads)."""
                po = h * DH
                for ib in range(NIB):
                    ptile = pt_pool.tile([P, JC, IB], BF16, tag="pt", name="ptile")
                    for jg in range(JC // 2):
                        ps_st = st_psum.tile(
                            [P, 2, IB], FP32, tag="st", name="st_ps"
                        )
                        for u in range(2):
                            jc = jg * 2 + u
                            nc.tensor.matmul(
                                ps_st[:, u, :],
                                kT[po : po + DH, b, jc * P : (jc + 1) * P],
                                qT[po : po + DH, b, ib * IB : (ib + 1) * IB],
                                start=True,
                                stop=True,
                            )
                        nc.scalar.activation(
                            ptile[:, jg * 2 : (jg + 1) * 2, :],
                            ps_st,
                            mybir.ActivationFunctionType.Exp,
                            scale=SCALE,
                        )
                    ps_o4 = o_psum.tile(
                        [P, ISUB, DH + 1], FP32, tag="po", name="o_ps"
                    )
                    for isub in range(ISUB):
                        for jc in range(JC):
                            nc.tensor.matmul(
                                ps_o4[:, isub, :],
                                ptile[:, jc, isub * P : (isub + 1) * P],
                                v_aug[:, b, jc, h, :],
                                start=(jc == 0),
                                stop=(jc == JC - 1),
                            )
                    for isub in range(ISUB):
                        ic = ib * ISUB + isub
                        recip = nrm.tile([P, 1], FP32, tag="recip", name="recip")
                        nc.vector.reciprocal(recip, ps_o4[:, isub, DH : DH + 1])
                        nc.vector.tensor_scalar_mul(
                            out_rows[:, b, ic, po : po + DH],
                            ps_o4[:, isub, 0:DH],
                            recip,
                        )
                    # block (h, b, ib) complete -> stage its A2A input
                    s = b * NIB + ib
                    nc.sync.dma_start(
                        a2a_ins[h][s].rearrange("ic p c -> p ic c"),
                        out_rows[
                            :, b, ib * ISUB : (ib + 1) * ISUB, po : po + DH
                        ],
                    )

            def a2a_exchange(h):
                nc.gpsimd.collective_compute(
                    "AllToAll",
                    mybir.AluOpType.bypass,
                    replica_groups=REPLICA_GROUPS,
                    ins=[a2a_ins[h].opt()],
                    outs=[a2a_outs[h].opt()],
                )

            def receive(h):
                po = h * DH
                for i in range(NCORES):
                    rstage = pt_pool.tile(
                        [P, NIB, DH], BF16, tag="rstage", name="rstage"
                    )
                    nc.sync.dma_start(
                        rstage,
                        a2a_outs[h][i].rearrange("(ic p) c -> p ic c", p=P),
                    )
                    rps = st_psum.tile([DH, NIB, P], BF16, tag="st", name="r_ps")
                    for q in range(NIB):
                        nc.tensor.transpose(rps[:, q, :], rstage[:, q, :], ident_bf)
                    nc.vector.tensor_copy(attnT[po : po + DH, i, :], rps)

            def final_projection():
                for cc in range(KO):
                    psf2 = st_psum.tile([P, 2, IB], FP32, tag="st", name="f_ps")
                    ps_f = psf2[:, 0, :ROWS]
                    for ko in range(KO):
                        nc.tensor.matmul(
                            ps_f,
                            wo_sb[:, ko, cc * P : (cc + 1) * P],
                            attnT[:, ko, :],
                            start=(ko == 0),
                            stop=(ko == KO - 1),
                        )
                    of = stage.tile([P, ROWS], FP32, tag="of", name="of")
                    nc.vector.tensor_scalar_add(of, ps_f, bias_sb[:, cc : cc + 1])
                    nc.sync.dma_start(out_ext[cc * P : (cc + 1) * P, :], of)

            # ---- program order chosen so exp starts early and the PE
            # always has lower-priority projection work to fill gaps ----
            load_qkv_weights()
            xT0 = load_x_and_transpose(0)
            qkv_proj(0, xT0)
            attention(0, 0)
            xT1 = load_x_and_transpose(1)
            load_out_weights()
            qkv_proj(1, xT1)
            attention(0, 1)
            a2a_exchange(0)        # flies under head-1 attention
            attention(1, 0)
            attention(1, 1)
            receive(0)             # PE work during the head-1 A2A flight
            a2a_exchange(1)
            receive(1)
            final_projection()

    nc.finalize()
    return nc


def _get_nc():
    if "nc" not in _NC_CACHE:
        _NC_CACHE["nc"] = _build()
    return _NC_CACHE["nc"]


def kernel(**inputs) -> np.ndarray:
    import os

    import ml_dtypes

    global LAST_RESULTS

    bf16 = ml_dtypes.bfloat16
    x = np.asarray(inputs["x"], dtype=np.float32)
    W_qkv = np.asarray(inputs["W_qkv"], dtype=np.float32)
    W_out = np.asarray(inputs["W_out"], dtype=np.float32)
    b_out = np.ascontiguousarray(np.asarray(inputs["b_out"], dtype=np.float32))

    x_bf = np.ascontiguousarray(x.reshape(B * N, DIM).astype(bf16))
    wo_bf = np.ascontiguousarray(W_out.astype(bf16))
    wqkv_bf = W_qkv.astype(bf16)

    nc = _get_nc()

    in_maps = []
    for c in range(NCORES):
        in_maps.append(
            {
                "x": x_bf,
                "wq": np.ascontiguousarray(
                    wqkv_bf[:, 0 * INNER + c * SH : 0 * INNER + (c + 1) * SH]
                ),
                "wk": np.ascontiguousarray(
                    wqkv_bf[:, 1 * INNER + c * SH : 1 * INNER + (c + 1) * SH]
                ),
                "wv": np.ascontiguousarray(
                    wqkv_bf[:, 2 * INNER + c * SH : 2 * INNER + (c + 1) * SH]
                ),
                "wo": wo_bf,
                "bo": b_out,
            }
        )

    trace = os.environ.get("BASS_KERNEL_TRACE", "0") == "1"
    res = run_bass_kernel_spmd(
        nc, in_maps, core_ids=list(range(NCORES)), trace=trace
    )
    LAST_RESULTS = res

    y = np.empty((B, N, DIM), dtype=np.float32)
    for c in range(NCORES):
        b, r = c // 4, c % 4
        y[b, r * ROWS : (r + 1) * ROWS, :] = res.results[c]["out"].T
    return y
